# revision 5
# baseline (speedup 1.0000x reference)
"""AttnBlock (GroupNorm + single-head spatial attention + proj + residual)
for Trainium2, SPMD across 8 NeuronCores.

Sharding: data-parallel over batch (4 images) x 2-way split of query
positions per image => 8 cores.  Attention is computed per-image with the
full key/value set on every core, so there are no collectives.

v2: all large matmuls run as fp8(e4m3) DoubleRow (two contraction rows
per PE cell => 2x MAC throughput).  Numerics (validated vs reference in
fp32 simulation, rel err ~5e-3 at tolerance 2e-2):
  - GroupNorm is folded into the projections as before; the folded
    weights are quantized to e4m3 scaled x4 and x is quantized scaled
    x4, so q/k/v come out of PSUM scaled x16 (keeps every operand in
    e4m3's normal range; std(16q) ~ 16 vs max 240).
  - exp uses the ACT affine: exp(score_psum/4096 - 4); the -4 shift
    keeps e in [6e-5, ~8] well inside e4m3 range, and cancels in the
    softmax ratio.
  - The softmax denominator comes from a 16.0-valued extra column of
    the (x16-scaled) vT operand, so numerator and denominator are both
    x16 and the epilogue reciprocal cancels the scale exactly.
  - exp is issued over [128, 2x512] PSUM score pairs (two j-tiles per
    ACTIVATE) to halve the ACT per-instruction overhead; ACT is the
    co-bottleneck with the PE.
  - wproj folded into v (softmax rows sum to one), k bias dropped
    (j-constant in the softmax), q bias kept, all as in v1.
"""

import numpy as np

import concourse.bacc as bacc
import concourse.bass as bass
import concourse.mybir as mybir
import concourse.tile as tile
from concourse.tile import add_dep_helper
from concourse.bass_utils import run_bass_kernel_spmd

F32 = mybir.dt.float32
F32R = mybir.dt.float32r
BF16 = mybir.dt.bfloat16
FP8 = mybir.dt.float8e4
DR = mybir.MatmulPerfMode.DoubleRow

C = 256          # channels
HW = 4096        # spatial positions (64*64)
B = 4            # batch
NCORES = 8
IH = HW // 2     # query positions per core
P = 128          # partitions
NCC = C // P     # channel chunks (2)
IBLK = 512       # query i-block (scores moving free dim)
NIB = IH // IBLK # 4 i-blocks per core
NJT = HW // P    # 32 key tiles
NG = NJT // 2    # 16 j-tile pairs (DoubleRow groups)
EPS = 1e-6
EXP_SCALE = 1.0 / 4096.0   # 1/(16*16*16) : x16 q, x16 k, 1/16 softmax scale
EXP_BIAS = -4.0            # cancels in softmax; keeps e4m3 in range
VCOL = 272       # vT tile free stride (pad 258 -> 272 for 16B ko-step rule)

_PROGRAM = None  # cached (nc)
LAST_RESULTS = None  # BassKernelResults of the most recent run (for test harness)
TRACE = False


def _round_f32r(x):
    u = np.ascontiguousarray(x, dtype=np.float32).view(np.uint32)
    r = ((u.astype(np.uint64) + 0x800) & 0xFFFFF000).astype(np.uint32)
    return r.view(np.float32)


def _build_program(reps=1):
    nc = bacc.Bacc()

    xr_d = nc.declare_dram_parameter("xr", [C, HW], F32R, isOutput=False)
    xth_d = nc.declare_dram_parameter("xth", [IH, C], F32, isOutput=False)
    wq_d = nc.declare_dram_parameter("wqt", [C, C], F32R, isOutput=False)
    wk_d = nc.declare_dram_parameter("wkt", [C, C], F32R, isOutput=False)
    w2_d = nc.declare_dram_parameter("w2t", [C, C], F32R, isOutput=False)
    bq_d = nc.declare_dram_parameter("bq16", [C], F32, isOutput=False)
    b2h_d = nc.declare_dram_parameter("b2h16", [C], F32, isOutput=False)  # 16*(wproj@bv+bproj)
    gns_d = nc.declare_dram_parameter("gns", [C], F32, isOutput=False)
    gnb_d = nc.declare_dram_parameter("gnb", [C], F32, isOutput=False)
    out_d = nc.declare_dram_parameter("out", [IH, C], F32, isOutput=True)

    b2_dram = nc.dram_tensor("b2_bounce", [C], F32)

    with tile.TileContext(nc) as tc:
      for _rep in range(reps):
        with (
            tc.tile_pool(name="wt", bufs=1) as wt,
            tc.tile_pool(name="xp", bufs=1) as xp,
            tc.tile_pool(name="qkv", bufs=1) as qkv,
            tc.tile_pool(name="scr", bufs=2) as scr,
        ):
            # ---------- constants ----------
            G = wt.tile([P, P], F32, tag="G", name="G")
            nc.gpsimd.memset(G, 0.0)
            nc.gpsimd.memset(G[0:64, 0:64], 1.0 / 64.0)
            nc.gpsimd.memset(G[64:128, 64:128], 1.0 / 64.0)
            eps_t = wt.tile([P, 1], F32, tag="eps", name="eps")
            nc.vector.memset(eps_t, EPS)
            ebias_t = wt.tile([P, 1], F32, tag="ebias", name="ebias")
            nc.vector.memset(ebias_t, EXP_BIAS)

            # ---------- x loads first (startup critical path) ----------
            xr_sb = [xp.tile([P, HW], F32R, tag=f"xr{cc}", name=f"xr{cc}")
                     for cc in range(NCC)]
            _eng = [nc.sync, nc.scalar, nc.gpsimd]
            for w in range(8):
                for cc in range(NCC):
                    _eng[(w * NCC + cc) % 3].dma_start(
                        out=xr_sb[cc][:, w * 512:(w + 1) * 512],
                        in_=xr_d[cc * P:(cc + 1) * P, w * 512:(w + 1) * 512],
                    )

            # ---------- load weights / params ----------
            w_sb = {}
            for name, d in (("q", wq_d), ("k", wk_d), ("v", w2_d)):
                for cc in range(NCC):
                    t = wt.tile([P, C], F32R, tag=f"w{name}{cc}", name=f"w{name}{cc}")
                    nc.scalar.dma_start(out=t, in_=d[cc * P:(cc + 1) * P, :])
                    w_sb[name, cc] = t
            par_sb = {}
            for name, d in (("bq16", bq_d), ("gns", gns_d), ("gnb", gnb_d)):
                for cc in range(NCC):
                    t = wt.tile([P, 1], F32, tag=f"{name}{cc}", name=f"{name}{cc}")
                    nc.scalar.dma_start(out=t, in_=d[cc * P:(cc + 1) * P].unsqueeze(1))
                    par_sb[name, cc] = t
            b2h_sb = wt.tile([1, C], F32, tag="b2h", name="b2h")
            nc.sync.dma_start(out=b2h_sb, in_=b2h_d[:].unsqueeze(0))

            # ---------- residual (needed only at epilogue; last in DMA order) ----------
            xth_sb = xp.tile([P, IH // P, C], F32, tag="xth", name="xth")
            xth_dmas = []
            for s in range(IH // P):
                xth_dmas.append(nc.sync.dma_start(out=xth_sb[:, s, :], in_=xth_d[s * P:(s + 1) * P, :]))

            # ---------- x -> fp8 (x4) in DoubleRow-paired layout ----------
            x8 = xp.tile([P, NCC, HW], FP8, tag="x8", name="x8")
            for w in range(8):
                for cc in range(NCC):
                    sl = slice(w * 512, (w + 1) * 512)
                    nc.gpsimd.tensor_scalar_mul(x8[:, cc, sl], xr_sb[cc][:, sl], 4.0)

            # ---------- GroupNorm stats (on rounded x; error ~1e-7) ----------
            with tc.tile_pool(name="psA", bufs=2, space="PSUM") as psA:
                # PE warm-up while x DMA + stats run
                warm_ps = psA.tile([P, 128], F32, tag="warm", name="warm")
                warm_rhs = wt.tile([P, 128], F32, tag="warm_rhs", name="warm_rhs")
                nc.gpsimd.memset(warm_rhs, 0.0)
                for _ in range(36):
                    nc.tensor.matmul(warm_ps, G, warm_rhs, start=True, stop=True)
                a4_sb, b_sb = [], []
                st6s = [scr.tile([P, 8, 6], F32, tag=f"st6{cc}", name=f"st6{cc}")
                        for cc in range(NCC)]
                last_bn = None
                for w in range(8):
                    for cc in range(NCC):
                        last_bn = nc.vector.bn_stats(out=st6s[cc][:, w, :], in_=xr_sb[cc][:, w * 512:(w + 1) * 512])
                for _d in xth_dmas:
                    add_dep_helper(_d.ins, last_bn.ins, sync=True,
                                   reason="defer residual load until stats read x")
                for cc in range(NCC):
                    st6 = st6s[cc]
                    mv = scr.tile([P, 2], F32, tag="mv", name="mv")
                    nc.vector.bn_aggr(out=mv, in_=st6)
                    st3 = scr.tile([P, 3], F32, tag="st3", name="st3")
                    nc.vector.tensor_copy(st3[:, 0:2], mv)
                    nc.vector.tensor_mul(st3[:, 2:3], mv[:, 0:1], mv[:, 0:1])
                    gp = psA.tile([P, 3], F32, tag="gp", name="gp")
                    nc.tensor.matmul(gp, G, st3, start=True, stop=True)
                    # group stats, broadcast per channel: mean, E[var], E[mean^2]
                    gs = scr.tile([P, 3], F32, tag="gs", name="gs")
                    nc.vector.tensor_copy(gs, gp)
                    t1 = scr.tile([P, 1], F32, tag="t1", name="t1")
                    nc.vector.tensor_mul(t1, gs[:, 0:1], gs[:, 0:1])
                    vg = scr.tile([P, 1], F32, tag="vg", name="vg")
                    nc.vector.tensor_add(vg, gs[:, 1:2], gs[:, 2:3])
                    nc.vector.tensor_sub(vg, vg, t1)
                    sd = scr.tile([P, 1], F32, tag="sd", name="sd")
                    nc.scalar.activation(out=sd, in_=vg, func=mybir.ActivationFunctionType.Sqrt, bias=eps_t)
                    rstd = scr.tile([P, 1], F32, tag="rstd", name="rstd")
                    nc.vector.reciprocal(rstd, sd)
                    a_t = scr.tile([P, 1], F32, tag=f"a{cc}", name=f"a{cc}")
                    nc.vector.tensor_mul(a_t, rstd, par_sb["gns", cc])
                    a4_t = wt.tile([P, 1], F32, tag=f"a4{cc}", name=f"a4{cc}")
                    nc.vector.tensor_scalar_mul(a4_t, a_t, 4.0)
                    t2 = scr.tile([P, 1], F32, tag="t2", name="t2")
                    nc.vector.tensor_mul(t2, gs[:, 0:1], a_t)
                    bg = scr.tile([P, 1], F32, tag="bg", name="bg")
                    nc.vector.tensor_sub(bg, par_sb["gnb", cc], t2)
                    b_t = wt.tile([P, 1], F32R, tag=f"b{cc}", name=f"b{cc}")
                    nc.vector.tensor_scalar_mul(b_t, bg, 16.0)  # x16 GN beta
                    a4_sb.append(a4_t)
                    b_sb.append(b_t)

                for _ in range(20):
                    nc.tensor.matmul(warm_ps, G, warm_rhs, start=True, stop=True)

                # ---------- fold GroupNorm scale into fp8 weights (x4) ----------
                wf8 = {}
                for name in ("q", "k", "v"):
                    t = wt.tile([P, NCC, C], FP8, tag=f"wf8{name}", name=f"wf8{name}")
                    for cc in range(NCC):
                        nc.vector.tensor_scalar_mul(t[:, cc, :], w_sb[name, cc], a4_sb[cc])
                    wf8[name] = t

                # ---------- effective biases (x16 scale) ----------
                be = {}
                for cc in range(NCC):
                    bp = psA.tile([P, 1], F32, tag="bp", name="bp")
                    nc.tensor.matmul(bp, w_sb["q", 0][:, cc * P:(cc + 1) * P].bitcast(F32), b_sb[0].bitcast(F32), start=True, stop=False)
                    nc.tensor.matmul(bp, w_sb["q", 1][:, cc * P:(cc + 1) * P].bitcast(F32), b_sb[1].bitcast(F32), start=False, stop=True)
                    t = wt.tile([P, 1], F32, tag=f"beq{cc}", name=f"beq{cc}")
                    nc.vector.tensor_add(t, bp, par_sb["bq16", cc])
                    be["q", cc] = t
                b2p = psA.tile([1, C], F32, tag="b2p", name="b2p")
                nc.tensor.matmul(b2p, b_sb[0].bitcast(F32), w_sb["v", 0].bitcast(F32), start=True, stop=False)
                nc.tensor.matmul(b2p, b_sb[1].bitcast(F32), w_sb["v", 1].bitcast(F32), start=False, stop=True)
                b2row = wt.tile([1, C], F32, tag="b2row", name="b2row")
                nc.vector.tensor_add(b2row, b2p, b2h_sb)
                nc.sync.dma_start(out=b2_dram[:].unsqueeze(0), in_=b2row)
                b2bc = wt.tile([P, C], F32, tag="b2bc", name="b2bc")
                nc.sync.dma_start(
                    out=b2bc,
                    in_=bass.AP(tensor=b2_dram, offset=0, ap=[[0, P], [1, C]]),
                )

            # ---------- projections (all DoubleRow fp8) ----------
            q8 = qkv.tile([P, NCC, IH], FP8, tag="q8", name="q8")
            k8 = qkv.tile([P, NCC, HW], FP8, tag="k8", name="k8")
            vT8 = qkv.tile([P, NJT, VCOL], FP8, tag="vT8", name="vT8")
            # denominator column (16.0) + one zero pad col (moving slice is 0:258)
            nc.vector.memset(vT8[:, :, C:C + 1], 16.0)
            nc.vector.memset(vT8[:, :, C + 1:C + 2], 0.0)

            with tc.tile_pool(name="psB", bufs=3, space="PSUM") as psB:
                for cc in range(NCC):
                    wq_st = wf8["q"][:, 0:NCC, cc * P:(cc + 1) * P]
                    for ib in range(NIB):
                        pq = psB.tile([P, IBLK], F32, tag="pq", name="pq")
                        sl = slice(ib * IBLK, (ib + 1) * IBLK)
                        nc.tensor.matmul(pq, wq_st, x8[:, 0:NCC, sl], start=True, stop=True, perf_mode=DR)
                        nc.vector.tensor_scalar_add(q8[:, cc, sl], pq, be["q", cc])
                for cc in range(NCC):
                    wk_st = wf8["k"][:, 0:NCC, cc * P:(cc + 1) * P]
                    for ib in range(HW // IBLK):
                        pk = psB.tile([P, IBLK], F32, tag="pq", name="pq")
                        sl = slice(ib * IBLK, (ib + 1) * IBLK)
                        nc.tensor.matmul(pk, wk_st, x8[:, 0:NCC, sl], start=True, stop=True, perf_mode=DR)
                        # k's bias only adds a j-constant to each softmax row
                        nc.vector.tensor_copy(k8[:, cc, sl], pk)
                for jt in range(NJT):
                    pv = psB.tile([P, C], F32, tag="pv", name="pv")
                    nc.tensor.matmul(pv, x8[:, 0:NCC, jt * P:(jt + 1) * P], wf8["v"], start=True, stop=True, perf_mode=DR)
                    # b2 (x16) added into v'; softmax weights sum to 1 so this
                    # equals adding it after normalization
                    nc.vector.tensor_add(vT8[:, jt, 0:C], pv, b2bc)

            # ---------- attention ----------
            with (
                tc.tile_pool(name="psS", bufs=2, space="PSUM") as psS,
                tc.tile_pool(name="psAT", bufs=4, space="PSUM") as psAT,
                tc.tile_pool(name="eP", bufs=3) as eP,
                tc.tile_pool(name="oP", bufs=3) as oP,
                tc.tile_pool(name="rP", bufs=4) as rP,
            ):
                for ib in range(NIB):
                    isl = slice(ib * IBLK, (ib + 1) * IBLK)
                    nsub = IBLK // P
                    at = [psAT.tile([P, 258], F32, tag="at", name="at") for _ in range(nsub)]
                    sps = {}

                    def scores(g):
                        sp = psS.tile([P, 2, IBLK], F32, tag="sp", name="sp")
                        for m in range(2):
                            jt = 2 * g + m
                            nc.tensor.matmul(
                                sp[:, m, :], k8[:, 0:NCC, jt * P:(jt + 1) * P],
                                q8[:, 0:NCC, isl], start=True, stop=True, perf_mode=DR,
                            )
                        sps[g] = sp

                    scores(0)
                    scores(1)
                    for g in range(NG):
                        eT = eP.tile([P, 2, IBLK], FP8, tag="eT", name="eT")
                        nc.scalar.activation(out=eT, in_=sps.pop(g), func=mybir.ActivationFunctionType.Exp,
                                             scale=EXP_SCALE, bias=ebias_t)
                        if g + 2 < NG:
                            scores(g + 2)
                        for s in range(nsub):
                            nc.tensor.matmul(
                                at[s], eT[:, 0:2, s * P:(s + 1) * P], vT8[:, 2 * g:2 * g + 2, 0:258],
                                start=(g == 0), stop=(g == NG - 1), perf_mode=DR,
                            )
                    for s in range(nsub):
                        gidx = ib * nsub + s
                        rec = rP.tile([P, 1], F32, tag="rec", name="rec")
                        nc.vector.reciprocal(rec, at[s][:, C:C + 1])
                        ot = oP.tile([P, C], F32, tag="ot", name="ot")
                        nc.vector.tensor_scalar_mul(ot, at[s][:, 0:C], rec)
                        nc.gpsimd.tensor_add(ot, ot, xth_sb[:, gidx, :])
                        nc.sync.dma_start(out=out_d[gidx * P:(gidx + 1) * P, :], in_=ot)

    nc.finalize()
    return nc


def _get_program():
    global _PROGRAM
    if _PROGRAM is None:
        _PROGRAM = _build_program()
    return _PROGRAM


def kernel(x, gn_scale, gn_bias, wq, bq, wk, bk, wv, bv, wproj, bproj):
    global LAST_RESULTS
    x = np.asarray(x, dtype=np.float32)
    gn_scale = np.asarray(gn_scale, dtype=np.float32)
    gn_bias = np.asarray(gn_bias, dtype=np.float32)
    wq_ = np.asarray(wq, dtype=np.float32)
    wk_ = np.asarray(wk, dtype=np.float32)
    wv_ = np.asarray(wv, dtype=np.float32)
    wp_ = np.asarray(wproj, dtype=np.float32)
    bq_ = np.asarray(bq, dtype=np.float32)
    bv_ = np.asarray(bv, dtype=np.float32)
    bp_ = np.asarray(bproj, dtype=np.float32)

    b, c, h, w = x.shape
    assert (b, c, h * w) == (B, C, HW), x.shape

    w2 = (wp_.astype(np.float64) @ wv_.astype(np.float64)).astype(np.float32)
    b2h16 = 16.0 * ((wp_.astype(np.float64) @ bv_.astype(np.float64)).astype(np.float32) + bp_)
    bq16 = 16.0 * bq_

    wqt = _round_f32r(np.ascontiguousarray(wq_.T))
    wkt = _round_f32r(np.ascontiguousarray(wk_.T))
    w2t = _round_f32r(np.ascontiguousarray(w2.T))

    xf = x.reshape(B, C, HW)
    in_maps = []
    for core in range(NCORES):
        bi, hi = core // 2, core % 2
        xi = np.roll(xf[bi], -IH * hi, axis=1)
        in_maps.append({
            "xr": _round_f32r(xi),
            "xth": np.ascontiguousarray(xi[:, :IH].T),
            "wqt": wqt, "wkt": wkt, "w2t": w2t,
            "bq16": bq16, "b2h16": b2h16,
            "gns": gn_scale, "gnb": gn_bias,
        })

    nc = _get_program()
    res = run_bass_kernel_spmd(nc, in_maps, list(range(NCORES)), trace=TRACE)
    LAST_RESULTS = res

    out = np.empty((B, C, HW), dtype=np.float32)
    for core in range(NCORES):
        bi, hi = core // 2, core % 2
        out[bi][:, hi * IH:(hi + 1) * IH] = res.results[core]["out"].T
    return out.reshape(B, C, h, w)


# revision 12
# speedup vs baseline: 2.0466x; 2.0466x over previous
"""AttnBlock (GroupNorm + single-head spatial attention + proj + residual)
for Trainium2, SPMD across 8 NeuronCores.

Sharding: data-parallel over batch (4 images) x 2-way split of query
positions per image => 8 cores.  Attention is computed per-image with the
full key/value set on every core, so there are no collectives.

v2: all large matmuls run as fp8(e4m3) DoubleRow (two contraction rows
per PE cell => 2x MAC throughput).  Numerics (validated vs reference in
fp32 simulation, rel err ~5e-3 at tolerance 2e-2):
  - GroupNorm is folded into the projections as before; the folded
    weights are quantized to e4m3 scaled x4 and x is quantized scaled
    x4, so q/k/v come out of PSUM scaled x16 (keeps every operand in
    e4m3's normal range; std(16q) ~ 16 vs max 240).
  - exp uses the ACT affine: exp(score_psum/4096 - 4); the -4 shift
    keeps e in [6e-5, ~8] well inside e4m3 range, and cancels in the
    softmax ratio.
  - The softmax denominator comes from a 16.0-valued extra column of
    the (x16-scaled) vT operand, so numerator and denominator are both
    x16 and the epilogue reciprocal cancels the scale exactly.
  - exp is issued over [128, 2x512] PSUM score pairs (two j-tiles per
    ACTIVATE) to halve the ACT per-instruction overhead; ACT is the
    co-bottleneck with the PE.
  - wproj folded into v (softmax rows sum to one), k bias dropped
    (j-constant in the softmax), q bias kept, all as in v1.
"""

import numpy as np

import concourse.bacc as bacc
import concourse.bass as bass
import concourse.mybir as mybir
import concourse.tile as tile
from concourse.tile import add_dep_helper
from concourse.bass_utils import run_bass_kernel_spmd

F32 = mybir.dt.float32
F32R = mybir.dt.float32r
BF16 = mybir.dt.bfloat16
FP8 = mybir.dt.float8e4
DR = mybir.MatmulPerfMode.DoubleRow

C = 256          # channels
HW = 4096        # spatial positions (64*64)
B = 4            # batch
NCORES = 8
IH = HW // 2     # query positions per core
P = 128          # partitions
NCC = C // P     # channel chunks (2)
IBLK = 512       # query i-block (scores moving free dim)
NIB = IH // IBLK # 4 i-blocks per core
NJT = HW // P    # 32 key tiles
NG = NJT // 2    # 16 j-tile pairs (DoubleRow groups)
EPS = 1e-6
EXP_SCALE = 1.0 / 4096.0   # 1/(16*16*16) : x16 q, x16 k, 1/16 softmax scale
EXP_BIAS = -4.0            # cancels in softmax; keeps e4m3 in range
VCOL = 272       # vT tile free stride (pad 258 -> 272 for 16B ko-step rule)

_PROGRAM = None  # cached (nc)
LAST_RESULTS = None  # BassKernelResults of the most recent run (for test harness)
TRACE = False


def _round_f32r(x):
    u = np.ascontiguousarray(x, dtype=np.float32).view(np.uint32)
    r = ((u.astype(np.uint64) + 0x800) & 0xFFFFF000).astype(np.uint32)
    return r.view(np.float32)


def _build_program(reps=1):
    nc = bacc.Bacc()

    xr_d = nc.declare_dram_parameter("xr", [C, HW], F32R, isOutput=False)
    xth_d = nc.declare_dram_parameter("xth", [IH, C], F32, isOutput=False)
    wq_d = nc.declare_dram_parameter("wqt", [C, C], F32R, isOutput=False)
    wk_d = nc.declare_dram_parameter("wkt", [C, C], F32R, isOutput=False)
    w2_d = nc.declare_dram_parameter("w2t", [C, C], F32R, isOutput=False)
    bq_d = nc.declare_dram_parameter("bq16", [C], F32, isOutput=False)
    b2h_d = nc.declare_dram_parameter("b2h16", [C], F32, isOutput=False)  # 16*(wproj@bv+bproj)
    gns_d = nc.declare_dram_parameter("gns", [C], F32, isOutput=False)
    gnb_d = nc.declare_dram_parameter("gnb", [C], F32, isOutput=False)
    out_d = nc.declare_dram_parameter("out", [IH, C], F32, isOutput=True)

    b2_dram = nc.dram_tensor("b2_bounce", [C], F32)

    with tile.TileContext(nc) as tc:
      for _rep in range(reps):
        with (
            tc.tile_pool(name="wt", bufs=1) as wt,
            tc.tile_pool(name="xp", bufs=1) as xp,
            tc.tile_pool(name="qkv", bufs=1) as qkv,
            tc.tile_pool(name="scr", bufs=2) as scr,
        ):
            # ---------- constants ----------
            G = wt.tile([P, P], F32, tag="G", name="G")
            nc.gpsimd.memset(G, 0.0)
            nc.gpsimd.memset(G[0:64, 0:64], 1.0 / 64.0)
            nc.gpsimd.memset(G[64:128, 64:128], 1.0 / 64.0)
            eps_t = wt.tile([P, 1], F32, tag="eps", name="eps")
            nc.vector.memset(eps_t, EPS)
            ebias_t = wt.tile([P, 1], F32, tag="ebias", name="ebias")
            nc.vector.memset(ebias_t, EXP_BIAS)

            # ---------- x loads first (startup critical path) ----------
            xr_sb = [xp.tile([P, HW], F32R, tag=f"xr{cc}", name=f"xr{cc}")
                     for cc in range(NCC)]
            _eng = [nc.sync, nc.scalar, nc.gpsimd]
            for w in range(8):
                for cc in range(NCC):
                    _eng[(w * NCC + cc) % 3].dma_start(
                        out=xr_sb[cc][:, w * 512:(w + 1) * 512],
                        in_=xr_d[cc * P:(cc + 1) * P, w * 512:(w + 1) * 512],
                    )

            # ---------- load weights / params ----------
            w_sb = {}
            for name, d in (("q", wq_d), ("k", wk_d), ("v", w2_d)):
                for cc in range(NCC):
                    t = wt.tile([P, C], F32R, tag=f"w{name}{cc}", name=f"w{name}{cc}")
                    nc.scalar.dma_start(out=t, in_=d[cc * P:(cc + 1) * P, :])
                    w_sb[name, cc] = t
            par_sb = {}
            for name, d in (("bq16", bq_d), ("gns", gns_d), ("gnb", gnb_d)):
                for cc in range(NCC):
                    t = wt.tile([P, 1], F32, tag=f"{name}{cc}", name=f"{name}{cc}")
                    nc.scalar.dma_start(out=t, in_=d[cc * P:(cc + 1) * P].unsqueeze(1))
                    par_sb[name, cc] = t
            b2h_sb = wt.tile([1, C], F32, tag="b2h", name="b2h")
            nc.sync.dma_start(out=b2h_sb, in_=b2h_d[:].unsqueeze(0))

            # ---------- residual (needed only at epilogue; last in DMA order) ----------
            xth_sb = xp.tile([P, IH // P, C], F32, tag="xth", name="xth")
            xth_dmas = []
            for s in range(IH // P):
                xth_dmas.append(nc.sync.dma_start(out=xth_sb[:, s, :], in_=xth_d[s * P:(s + 1) * P, :]))

            # ---------- x -> fp8 (x4) in DoubleRow-paired layout ----------
            x8 = xp.tile([P, NCC, HW], FP8, tag="x8", name="x8")
            for w in range(8):
                for cc in range(NCC):
                    sl = slice(w * 512, (w + 1) * 512)
                    nc.vector.tensor_scalar_mul(x8[:, cc, sl], xr_sb[cc][:, sl], 4.0)

            # ---------- GroupNorm stats (on rounded x; error ~1e-7) ----------
            with tc.tile_pool(name="psA", bufs=2, space="PSUM") as psA:
                # PE warm-up while x DMA + stats run (bf16: cheap per-MM)
                warm_ps = psA.tile([P, 512], F32, tag="warm", name="warm")
                warm_w = wt.tile([P, 128], BF16, tag="warm_w", name="warm_w")
                warm_rhs = wt.tile([P, 512], BF16, tag="warm_rhs", name="warm_rhs")
                nc.gpsimd.memset(warm_w, 0.0)
                nc.gpsimd.memset(warm_rhs, 0.0)
                for _ in range(36):
                    nc.tensor.matmul(warm_ps, warm_w, warm_rhs, start=True, stop=True)
                a4_sb, b_sb = [], []
                st6s = [scr.tile([P, 8, 6], F32, tag=f"st6{cc}", name=f"st6{cc}")
                        for cc in range(NCC)]
                last_bn = None
                for w in range(8):
                    for cc in range(NCC):
                        last_bn = nc.vector.bn_stats(out=st6s[cc][:, w, :], in_=xr_sb[cc][:, w * 512:(w + 1) * 512])
                for _d in xth_dmas:
                    add_dep_helper(_d.ins, last_bn.ins, sync=True,
                                   reason="defer residual load until stats read x")
                for cc in range(NCC):
                    st6 = st6s[cc]
                    mv = scr.tile([P, 2], F32, tag="mv", name="mv")
                    nc.vector.bn_aggr(out=mv, in_=st6)
                    st3 = scr.tile([P, 3], F32, tag="st3", name="st3")
                    nc.vector.tensor_copy(st3[:, 0:2], mv)
                    nc.vector.tensor_mul(st3[:, 2:3], mv[:, 0:1], mv[:, 0:1])
                    gp = psA.tile([P, 3], F32, tag="gp", name="gp")
                    nc.tensor.matmul(gp, G, st3, start=True, stop=True)
                    # group stats, broadcast per channel: mean, E[var], E[mean^2]
                    gs = scr.tile([P, 3], F32, tag="gs", name="gs")
                    nc.vector.tensor_copy(gs, gp)
                    t1 = scr.tile([P, 1], F32, tag="t1", name="t1")
                    nc.vector.tensor_mul(t1, gs[:, 0:1], gs[:, 0:1])
                    vg = scr.tile([P, 1], F32, tag="vg", name="vg")
                    nc.vector.tensor_add(vg, gs[:, 1:2], gs[:, 2:3])
                    nc.vector.tensor_sub(vg, vg, t1)
                    sd = scr.tile([P, 1], F32, tag="sd", name="sd")
                    nc.scalar.activation(out=sd, in_=vg, func=mybir.ActivationFunctionType.Sqrt, bias=eps_t)
                    rstd = scr.tile([P, 1], F32, tag="rstd", name="rstd")
                    nc.vector.reciprocal(rstd, sd)
                    a_t = scr.tile([P, 1], F32, tag=f"a{cc}", name=f"a{cc}")
                    nc.vector.tensor_mul(a_t, rstd, par_sb["gns", cc])
                    a4_t = wt.tile([P, 1], F32, tag=f"a4{cc}", name=f"a4{cc}")
                    nc.vector.tensor_scalar_mul(a4_t, a_t, 4.0)
                    t2 = scr.tile([P, 1], F32, tag="t2", name="t2")
                    nc.vector.tensor_mul(t2, gs[:, 0:1], a_t)
                    bg = scr.tile([P, 1], F32, tag="bg", name="bg")
                    nc.vector.tensor_sub(bg, par_sb["gnb", cc], t2)
                    b_t = wt.tile([P, 1], F32R, tag=f"b{cc}", name=f"b{cc}")
                    nc.vector.tensor_scalar_mul(b_t, bg, 16.0)  # x16 GN beta
                    a4_sb.append(a4_t)
                    b_sb.append(b_t)

                for _ in range(20):
                    nc.tensor.matmul(warm_ps, warm_w, warm_rhs, start=True, stop=True)

                # ---------- fold GroupNorm scale into fp8 weights (x4) ----------
                wf8 = {}
                for name in ("q", "k", "v"):
                    t = wt.tile([P, NCC, C], FP8, tag=f"wf8{name}", name=f"wf8{name}")
                    for cc in range(NCC):
                        nc.vector.tensor_scalar_mul(t[:, cc, :], w_sb[name, cc], a4_sb[cc])
                    wf8[name] = t

                # ---------- effective biases (x16 scale) ----------
                be = {}
                for cc in range(NCC):
                    bp = psA.tile([P, 1], F32, tag="bp", name="bp")
                    nc.tensor.matmul(bp, w_sb["q", 0][:, cc * P:(cc + 1) * P].bitcast(F32), b_sb[0].bitcast(F32), start=True, stop=False)
                    nc.tensor.matmul(bp, w_sb["q", 1][:, cc * P:(cc + 1) * P].bitcast(F32), b_sb[1].bitcast(F32), start=False, stop=True)
                    t = wt.tile([P, 1], F32, tag=f"beq{cc}", name=f"beq{cc}")
                    nc.vector.tensor_add(t, bp, par_sb["bq16", cc])
                    be["q", cc] = t
                b2p = psA.tile([1, C], F32, tag="b2p", name="b2p")
                nc.tensor.matmul(b2p, b_sb[0].bitcast(F32), w_sb["v", 0].bitcast(F32), start=True, stop=False)
                nc.tensor.matmul(b2p, b_sb[1].bitcast(F32), w_sb["v", 1].bitcast(F32), start=False, stop=True)
                b2row = wt.tile([1, C], F32, tag="b2row", name="b2row")
                nc.vector.tensor_add(b2row, b2p, b2h_sb)
                nc.sync.dma_start(out=b2_dram[:].unsqueeze(0), in_=b2row)
                b2bc = wt.tile([P, C], F32, tag="b2bc", name="b2bc")
                nc.sync.dma_start(
                    out=b2bc,
                    in_=bass.AP(tensor=b2_dram, offset=0, ap=[[0, P], [1, C]]),
                )

            # ---------- projections (all DoubleRow fp8) ----------
            # q8 pair-interleaved: element (cc, i) at free offset 2*i+cc so the
            # DoubleRow moving pair is adjacent in SBUF (single read per col)
            q8 = qkv.tile([P, IH, NCC], FP8, tag="q8", name="q8")
            k8 = qkv.tile([P, NCC, HW], FP8, tag="k8", name="k8")
            vT8 = qkv.tile([P, NJT, VCOL], FP8, tag="vT8", name="vT8")
            # denominator column (16.0) + one zero pad col (moving slice is 0:258)
            nc.vector.memset(vT8[:, :, C:C + 1], 16.0)
            nc.vector.memset(vT8[:, :, C + 1:C + 2], 0.0)

            with tc.tile_pool(name="psB", bufs=3, space="PSUM") as psB:
                for cc in range(NCC):
                    wq_st = wf8["q"][:, 0:NCC, cc * P:(cc + 1) * P]
                    for ib in range(NIB):
                        pq = psB.tile([P, IBLK], F32, tag="pq", name="pq")
                        sl = slice(ib * IBLK, (ib + 1) * IBLK)
                        nc.tensor.matmul(pq, wq_st, x8[:, 0:NCC, sl], start=True, stop=True, perf_mode=DR)
                        nc.vector.tensor_scalar_add(q8[:, sl, cc], pq, be["q", cc])
                for cc in range(NCC):
                    wk_st = wf8["k"][:, 0:NCC, cc * P:(cc + 1) * P]
                    for ib in range(HW // IBLK):
                        pk = psB.tile([P, IBLK], F32, tag="pq", name="pq")
                        sl = slice(ib * IBLK, (ib + 1) * IBLK)
                        nc.tensor.matmul(pk, wk_st, x8[:, 0:NCC, sl], start=True, stop=True, perf_mode=DR)
                        # k's bias only adds a j-constant to each softmax row
                        nc.vector.tensor_copy(k8[:, cc, sl], pk)
                for jt in range(NJT):
                    pv = psB.tile([P, C], F32, tag="pv", name="pv")
                    nc.tensor.matmul(pv, x8[:, 0:NCC, jt * P:(jt + 1) * P], wf8["v"], start=True, stop=True, perf_mode=DR)
                    # b2 (x16) added into v'; softmax weights sum to 1 so this
                    # equals adding it after normalization
                    nc.vector.tensor_add(vT8[:, jt, 0:C], pv, b2bc)

            # ---------- attention ----------
            with (
                tc.tile_pool(name="psS", bufs=2, space="PSUM") as psS,
                tc.tile_pool(name="psAT", bufs=4, space="PSUM") as psAT,
                tc.tile_pool(name="eP", bufs=3) as eP,
                tc.tile_pool(name="oP", bufs=3) as oP,
                tc.tile_pool(name="rP", bufs=4) as rP,
            ):
                for ib in range(NIB):
                    isl = slice(ib * IBLK, (ib + 1) * IBLK)
                    nsub = IBLK // P
                    at = [psAT.tile([P, 258], F32, tag="at", name="at") for _ in range(nsub)]
                    sps = {}

                    def scores(g):
                        sp = psS.tile([P, 2, IBLK], F32, tag="sp", name="sp")
                        for m in range(2):
                            jt = 2 * g + m
                            nc.tensor.matmul(
                                sp[:, m, :], k8[:, 0:NCC, jt * P:(jt + 1) * P],
                                q8[:, isl, 0:NCC].transpose([0, 2, 1]),
                                start=True, stop=True, perf_mode=DR,
                            )
                        sps[g] = sp

                    scores(0)
                    scores(1)
                    for g in range(NG):
                        eT = eP.tile([P, 2, IBLK], FP8, tag="eT", name="eT")
                        nc.scalar.activation(out=eT, in_=sps.pop(g), func=mybir.ActivationFunctionType.Exp,
                                             scale=EXP_SCALE, bias=ebias_t)
                        if g + 2 < NG:
                            scores(g + 2)
                        for s in range(nsub):
                            nc.tensor.matmul(
                                at[s], eT[:, 0:2, s * P:(s + 1) * P], vT8[:, 2 * g:2 * g + 2, 0:258],
                                start=(g == 0), stop=(g == NG - 1), perf_mode=DR,
                            )
                    for s in range(nsub):
                        gidx = ib * nsub + s
                        rec = rP.tile([P, 1], F32, tag="rec", name="rec")
                        nc.vector.reciprocal(rec, at[s][:, C:C + 1])
                        ot = oP.tile([P, C], F32, tag="ot", name="ot")
                        nc.vector.tensor_scalar_mul(ot, at[s][:, 0:C], rec)
                        nc.vector.tensor_add(ot, ot, xth_sb[:, gidx, :])
                        nc.sync.dma_start(out=out_d[gidx * P:(gidx + 1) * P, :], in_=ot)

    nc.finalize()
    return nc


def _get_program():
    global _PROGRAM
    if _PROGRAM is None:
        _PROGRAM = _build_program()
    return _PROGRAM


def kernel(x, gn_scale, gn_bias, wq, bq, wk, bk, wv, bv, wproj, bproj):
    global LAST_RESULTS
    x = np.asarray(x, dtype=np.float32)
    gn_scale = np.asarray(gn_scale, dtype=np.float32)
    gn_bias = np.asarray(gn_bias, dtype=np.float32)
    wq_ = np.asarray(wq, dtype=np.float32)
    wk_ = np.asarray(wk, dtype=np.float32)
    wv_ = np.asarray(wv, dtype=np.float32)
    wp_ = np.asarray(wproj, dtype=np.float32)
    bq_ = np.asarray(bq, dtype=np.float32)
    bv_ = np.asarray(bv, dtype=np.float32)
    bp_ = np.asarray(bproj, dtype=np.float32)

    b, c, h, w = x.shape
    assert (b, c, h * w) == (B, C, HW), x.shape

    w2 = (wp_.astype(np.float64) @ wv_.astype(np.float64)).astype(np.float32)
    b2h16 = 16.0 * ((wp_.astype(np.float64) @ bv_.astype(np.float64)).astype(np.float32) + bp_)
    bq16 = 16.0 * bq_

    wqt = _round_f32r(np.ascontiguousarray(wq_.T))
    wkt = _round_f32r(np.ascontiguousarray(wk_.T))
    w2t = _round_f32r(np.ascontiguousarray(w2.T))

    xf = x.reshape(B, C, HW)
    in_maps = []
    for core in range(NCORES):
        bi, hi = core // 2, core % 2
        xi = np.roll(xf[bi], -IH * hi, axis=1)
        in_maps.append({
            "xr": _round_f32r(xi),
            "xth": np.ascontiguousarray(xi[:, :IH].T),
            "wqt": wqt, "wkt": wkt, "w2t": w2t,
            "bq16": bq16, "b2h16": b2h16,
            "gns": gn_scale, "gnb": gn_bias,
        })

    nc = _get_program()
    res = run_bass_kernel_spmd(nc, in_maps, list(range(NCORES)), trace=TRACE)
    LAST_RESULTS = res

    out = np.empty((B, C, HW), dtype=np.float32)
    for core in range(NCORES):
        bi, hi = core // 2, core % 2
        out[bi][:, hi * IH:(hi + 1) * IH] = res.results[core]["out"].T
    return out.reshape(B, C, h, w)
